# revision 53
# baseline (speedup 1.0000x reference)
"""Multi-head attention (B=4, T=S=2048, E=1024, H=16, D=64) on 8 TRN2 NeuronCores.

Sharding: core c handles batch b=c//2 and head-group g=c%2 (8 of 16 heads).
Each core computes its 8 heads' attention plus the matching column-slice of
the output projection, producing a partial [T, E] f32 output. Host sums the
two partials per batch and adds bo.

On-chip dataflow (all matmuls bf16 with fp32 PSUM accumulation):
  qT[d,t] = WqT.T @ queryT       (d-major projections, per 128-dim head pair)
  kT[d,t] likewise; v[s,d] natural via value.T as the stationary operand
  S.T[s,t] = kT_h.T @ qT_h       (two heads row-packed in the 128-row PE array)
  expS.T   = exp(S.T * 1/8)      (ScalarE, PSUM -> SBUF bf16)
  O[t,d]   = expS.T.T @ v_h      (exp tile stationary, v moving: charges 64
                                  cycles/matmul instead of 512 -> PV at its
                                  cost-model floor; denominators via 1-wide
                                  matmuls against a ones column)
  Onorm    = (O * 1/den).T       (DVE per-partition scalar mul, then an
                                  SBUF->SBUF DMA-transpose back to [d, t])
  partial  = Onorm.T @ WoSlice   (accumulate over the core's 4 head pairs)

Emission is software-pipelined: stage s=(pair, t-quarter); each stage's 16
score-tile slots interleave the previous stage's PV at 2 PV-slots per score
slot (normalize runs mid-stage so the single PV-accumulator PSUM bank is
clear well before reuse) plus spread-out projection / v-projection /
out-projection work, keeping both ScalarE (exp) and PE continuously fed.
"""

from contextlib import ExitStack

import numpy as np
import ml_dtypes

B, T, S, E = 4, 2048, 2048, 1024
H, D = 16, 64
DC = 512          # dims per core (8 heads x 64)
NP = 4            # head pairs per core
NS = S // 128     # 16 s-tiles
NQ = 4            # t-quarters of 512

_BF16 = ml_dtypes.bfloat16

_cached = None


def _build(repeats=1):
    import concourse.bass as bass
    import concourse.mybir as mybir
    import concourse.tile as tile
    from concourse import bacc

    f32 = mybir.dt.float32
    bf16 = mybir.dt.bfloat16
    AF = mybir.ActivationFunctionType

    nc = bacc.Bacc("TRN2", target_bir_lowering=False)

    qT_d = nc.dram_tensor("qT", [E, T], bf16, kind="ExternalInput")
    kT_d = nc.dram_tensor("kT", [E, S], bf16, kind="ExternalInput")
    vT_d = nc.dram_tensor("vT", [E, S], bf16, kind="ExternalInput")
    # q/k projection weights arrive pre-tiled pair-major: Wq0 is pair 0's
    # [128, 8 e-chunks x 128] block (one small contiguous DMA on the
    # startup critical path), WqR the remaining three pairs.
    Wq0_d = nc.dram_tensor("Wq0", [128, 1024], bf16, kind="ExternalInput")
    WqR_d = nc.dram_tensor("WqR", [128, 3072], bf16, kind="ExternalInput")
    Wk0_d = nc.dram_tensor("Wk0", [128, 1024], bf16, kind="ExternalInput")
    WkR_d = nc.dram_tensor("WkR", [128, 3072], bf16, kind="ExternalInput")
    WvT_d = nc.dram_tensor("WvT", [E, DC], bf16, kind="ExternalInput")
    WoS_d = nc.dram_tensor("WoS", [DC, E], bf16, kind="ExternalInput")
    ident_d = nc.dram_tensor("ident", [128, 128], bf16, kind="ExternalInput")
    bqk_d = nc.dram_tensor("bqk", [128, 2 * NP], f32, kind="ExternalInput")
    bv_d = nc.dram_tensor("bv", [1, DC], f32, kind="ExternalInput")
    out_d = nc.dram_tensor("out", [T, E], f32, kind="ExternalOutput")

    with tile.TileContext(nc) as tc, ExitStack() as ctx:
        persist = ctx.enter_context(tc.tile_pool(name="persist", bufs=1))
        psc = ctx.enter_context(tc.tile_pool(name="psc", bufs=2, space="PSUM"))
        pacc = ctx.enter_context(tc.tile_pool(name="pacc", bufs=1, space="PSUM"))
        pden = ctx.enter_context(tc.tile_pool(name="pden", bufs=1, space="PSUM"))
        pmx = ctx.enter_context(tc.tile_pool(name="pmx", bufs=2, space="PSUM"))
        expool = ctx.enter_context(tc.tile_pool(name="expool", bufs=18))
        small = ctx.enter_context(tc.tile_pool(name="small", bufs=10))
        otp = ctx.enter_context(tc.tile_pool(name="otp", bufs=8))
        ocp_pool = ctx.enter_context(tc.tile_pool(name="ocp", bufs=3))
        xin = ctx.enter_context(tc.tile_pool(name="xin", bufs=8))
        wpool = ctx.enter_context(tc.tile_pool(name="wts", bufs=1))

        # ---- persistent SBUF tiles ----
        qTs = [persist.tile([128, T], bf16, tag=f"qT{p}", name=f"qT{p}") for p in range(NP)]
        kTs = [persist.tile([128, S], bf16, tag=f"kT{p}", name=f"kT{p}") for p in range(NP)]
        vts = [persist.tile([128, DC], bf16, tag=f"v{st}", name=f"v{st}") for st in range(NS)]
        WoSs = [persist.tile([128, E], bf16, tag=f"wo{p}", name=f"wo{p}") for p in range(NP)]
        Onorm = [persist.tile([128, T], bf16, tag=f"on{p}", name=f"on{p}") for p in range(NP)]
        bqk_sb = persist.tile([128, 2 * NP], f32, tag="bqk", name="bqk_sb")
        bq_sb = bqk_sb[:, 0:NP]
        bk_sb = bqk_sb[:, NP:2 * NP]
        bv_sb = persist.tile([128, DC], f32, tag="bv", name="bv_sb")
        ones_sb = persist.tile([128, 1], bf16, tag="ones", name="ones_sb")
        ident_sb = persist.tile([128, 128], bf16, tag="ident", name="ident_sb")
        # pair-major q/k weight walls: slice (p, e) at cols (p*8+e)*128
        wq2 = persist.tile([128, 4096], bf16, tag="wq2", name="wq2")
        wk2 = persist.tile([128, 4096], bf16, tag="wk2", name="wk2")

        nc.vector.memset(ones_sb, 1.0)

        def load_late_inputs():
            """Output-projection weights: first use ~stage 13; pin them
            past the projection-heavy first third of the timeline."""
            with tc.tile_wait_until(0.065):
                for p in range(NP):
                    nc.scalar.dma_start(out=WoSs[p],
                                        in_=WoS_d[p * 128:(p + 1) * 128, :])

        def load_wall(dram):
            """All 8 e-chunks of one weight set, as two strided DMAs:
            wall[:, e*DC + c] = dram[e*128 + p, c]."""
            t_ = wpool.tile([128, 8 * DC], bf16, tag="w", name="wall")
            for g in range(2):
                nc.sync.dma_start(
                    out=t_[:, g * 4 * DC:(g + 1) * 4 * DC
                           ].rearrange("p (e c) -> p e c", c=DC),
                    in_=dram[g * 512:(g + 1) * 512, :
                             ].rearrange("(e p) c -> p e c", p=128))
            return t_

        def proj_thunks(p, x_dram, wall, dst, bias_sb, eng=None):
            """One pair's q/k projection, quarter-granular: per t-quarter
            one xin tile carrying all 8 e-chunks ([128, 8x512]) loaded as
            two e-half DMAs, then 8 accumulating MMs in a pmx tile and a
            bias-add drain. Thunk layout: [open, mm03, mm47, drain] x 4
            quarters. eng picks the DMA-issue queue per quarter (SP
            default; Pool gives startup-critical loads their own DGE)."""
            engs = eng if isinstance(eng, list) else [eng or nc.sync] * 4
            thunks = []
            for q in range(4):
                xq = []

                def open_q(q=q, xq=xq, qeng=engs[q]):
                    # two e-half DMAs: halves the DMA_ENGINES blocking
                    # granularity (transposes/outputs queue behind these)
                    xt = xin.tile([128, 4096], bf16, tag="xin", name="xin")
                    v = xt.rearrange("p (e t) -> p e t", e=8)
                    for g in range(2):
                        qeng.dma_start(
                            out=v[:, g * 4:(g + 1) * 4, :],
                            in_=x_dram[g * 512:(g + 1) * 512,
                                       q * 512:(q + 1) * 512
                                       ].rearrange("(e p) t -> p e t",
                                                   p=128))
                    xq.append(xt)

                thunks.append(open_q)
                ps = []

                def mme(lo, hi, ps=ps, xq=xq):
                    if lo == 0:
                        ps.append(pmx.tile([128, 512], f32, tag="mx",
                                           name="mx_ps"))
                    for e in range(lo, hi):
                        nc.tensor.matmul(
                            ps[0],
                            wall[:, (p * 8 + e) * 128:(p * 8 + e + 1) * 128],
                            xq[0][:, e * 512:(e + 1) * 512],
                            start=(e == 0),
                            stop=(e == 7),
                        )

                thunks.append(lambda f=mme: f(0, 4))
                thunks.append(lambda f=mme: f(4, 8))

                def close_q(q=q, ps=ps, xq=xq):
                    nc.vector.tensor_scalar_add(
                        dst[:, q * 512:(q + 1) * 512],
                        ps[0], bias_sb[:, p:p + 1])
                    ps.clear()
                    xq.clear()

                thunks.append(close_q)
            return thunks

        def vproj_thunks(wv_tiles, dh):
            """V projection for head-quad dh (4 heads, N=256), streamed in
            four s-quarters: per quarter one xin DMA + 4 s-tile MM groups.
            dh=0 feeds pairs 0-1 (needed by stage 1); dh=1 feeds pairs 2-3
            (needed from stage 9). Thunks: [open, vst x4] x 4 quarters."""
            thunks = []
            for q in range(4):
                vq = []

                def open_q(q=q, vq=vq):
                    # v loads ride the idle Pool engine's SWDGE path: no
                    # HWDGE contention, keeps the SP sequencer free
                    vt = xin.tile([128, 4096], bf16, tag="xin", name="vxin")
                    v = vt.rearrange("p (e t) -> p e t", e=8)
                    for g in range(2):
                        nc.gpsimd.dma_start(
                            out=v[:, g * 4:(g + 1) * 4, :],
                            in_=vT_d[g * 512:(g + 1) * 512,
                                     q * 512:(q + 1) * 512
                                     ].rearrange("(e p) t -> p e t", p=128))
                    vq.append(vt)

                thunks.append(open_q)
                for sti in range(4):
                    def vst(sti=sti, q=q, vq=vq):
                        st = q * 4 + sti
                        ps = pmx.tile([128, 512], f32, tag="mx", name="mx_ps")
                        for e in range(8):
                            nc.tensor.matmul(
                                ps[:, 0:256],
                                vq[0][:, e * 512 + sti * 128:
                                      e * 512 + (sti + 1) * 128],
                                wv_tiles[:, e * DC + dh * 256:
                                         e * DC + (dh + 1) * 256],
                                start=(e == 0),
                                stop=(e == 7),
                            )
                        nc.vector.tensor_add(
                            vts[st][:, dh * 256:(dh + 1) * 256],
                            ps[:, 0:256],
                            bv_sb[:, dh * 256:(dh + 1) * 256],
                        )
                        if sti == 3:
                            vq.clear()
                    thunks.append(vst)
            return thunks

        def outproj_thunks(tq):
            thunks = []
            for tt in range(tq * 4, tq * 4 + 4):
                for c in range(2):
                    def unit(tt=tt, c=c):
                        op_ps = pmx.tile([128, 512], f32, tag="mx", name="mx_ps")
                        for p in range(NP):
                            nc.tensor.matmul(
                                op_ps,
                                Onorm[p][:, tt * 128:(tt + 1) * 128],
                                WoSs[p][:, c * 512:(c + 1) * 512],
                                start=(p == 0),
                                stop=(p == 3),
                            )
                        oc = ocp_pool.tile([128, 512], f32, tag="ocp", name="oc")
                        nc.vector.tensor_copy(oc, op_ps)
                        nc.sync.dma_start(
                            out=out_d[tt * 128:(tt + 1) * 128,
                                      c * 512:(c + 1) * 512],
                            in_=oc)
                    thunks.append(unit)
            return thunks

        class PrevStage:
            def __init__(self, p, tq, exs):
                self.p, self.tq, self.exs = p, tq, exs
                self.acc = None   # [128 t, 512]: 8 x 64-wide accums, idx 4h+k
                self.den = None   # [128 t, 16]: cols 2k+h
                self.ots = None   # normalized [t, 128] tiles awaiting transpose

        def emit_pv_slot(prev, st):
            """PV for one s-tile of the previous stage: per (head h, t-tile
            k), a 64-wide main matmul (exp stationary, v moving) plus a
            1-wide denominator matmul against the ones column. One PSUM
            accumulation group per bank (lazy per-byte init)."""
            for h in range(2):
                vsl = vts[st][:, (2 * prev.p + h) * 64:(2 * prev.p + h + 1) * 64]
                for k in range(4):
                    idx = 4 * h + k
                    exsl = prev.exs[st][:, h * 512 + k * 128:
                                        h * 512 + (k + 1) * 128]
                    first = (st == 0 and idx == 0)
                    last = (st == NS - 1 and idx == 7)
                    nc.tensor.matmul(
                        prev.acc[:, idx * 64:(idx + 1) * 64],
                        exsl, vsl,
                        start=first, stop=last,
                    )
                    nc.tensor.matmul(
                        prev.den[:, 2 * k + h:2 * k + h + 1],
                        exsl, ones_sb,
                        start=first, stop=last,
                    )

        def emit_normalize_muls(prev):
            """Phase A of normalizing the previous stage's PV accumulators:
            one reciprocal over all 8 denominators, then per t-tile k a
            scalar-multiply of both heads' 64 cols into a [t, 128] bf16
            tile (DVE only — PE keeps streaming)."""
            rc = small.tile([128, 8], f32, tag="rc", name="rc")
            nc.vector.reciprocal(rc, prev.den[:, 0:8])
            ots = []
            for k in range(4):
                ot = otp.tile([128, 128], bf16, tag="ot", name="ot")
                for h in range(2):
                    nc.vector.tensor_scalar_mul(
                        ot[:, h * 64:(h + 1) * 64],
                        prev.acc[:, (4 * h + k) * 64:(4 * h + k + 1) * 64],
                        rc[:, 2 * k + h:2 * k + h + 1],
                    )
                ots.append(ot)
            return ots

        def emit_normalize_tp(prev, k):
            """Phase B (spread one per slot so the pmx ring and DVE copies
            never block PE): PE row-column transpose of one [t, 128] tile
            through a pmx slot, DVE copy into the [d, t]-major Onorm
            slice. Keeps normalize off the DMA queues entirely."""
            t0 = prev.tq * 4
            # ride the pacc ring: the acc tile was already consumed by the
            # muls, and the next stage's acc alloc waits for tp3's copy —
            # keeps transposes out of the busy pmx ring
            tp = pacc.tile([128, 128], bf16, tag="acc", name="tp_ps")
            nc.tensor.transpose(tp, prev.ots[k], ident_sb)
            nc.vector.tensor_copy(
                Onorm[prev.p][:, (t0 + k) * 128:(t0 + k + 1) * 128], tp)

        def emit_stage(p, tq, prev, extras, dl=6, pv_start=0):
            """16 score slots for (p, tq); interleave prev stage's PV at two
            PV-slots per score slot from `pv_start` (all its exp tiles
            already exist), normalizing mid-stage so the single PV
            accumulator bank clears well before the next stage reuses it;
            extra thunks all emitted by slot `dl`. Returns this stage's
            PrevStage record."""
            t0 = tq * 512
            exs = []
            if prev is not None:
                prev.acc = pacc.tile([128, 512], f32, tag="acc", name="acc_ps")
                prev.den = pden.tile([128, 16], f32, tag="den", name="den_ps")
            n_ex = len(extras)
            taken = 0
            pv_done = 0
            for st in range(NS):
                sc_ps = psc.tile([128, 1024], f32, tag="sc", name="sc_ps")
                nc.tensor.matmul(
                    sc_ps[:, 0:512],
                    kTs[p][0:64, st * 128:(st + 1) * 128],
                    qTs[p][0:64, t0:t0 + 512],
                    start=True, stop=True,
                    tile_position=(0, 0),
                )
                nc.tensor.matmul(
                    sc_ps[:, 512:1024],
                    kTs[p][64:128, st * 128:(st + 1) * 128],
                    qTs[p][64:128, t0:t0 + 512],
                    start=True, stop=True,
                    tile_position=(64, 0),
                )
                ex = expool.tile([128, 1024], bf16, tag="ex", name="ex")
                nc.scalar.activation(ex, sc_ps, AF.Exp, scale=0.125)
                exs.append(ex)
                if prev is not None and st >= pv_start:
                    if NS + 1 < pv_done < NS + 6:
                        emit_normalize_tp(prev, pv_done - NS - 2)
                        pv_done += 1
                    elif pv_done == NS + 1:
                        pv_done += 1   # one settle slot for the DVE muls
                    for _ in range(2):
                        if pv_done < NS:
                            emit_pv_slot(prev, pv_done)
                            pv_done += 1
                    if pv_done == NS:
                        prev.ots = emit_normalize_muls(prev)
                        pv_done = NS + 1
                want = (n_ex * min(st + 1, dl) + dl - 1) // dl
                while taken < want:
                    extras[taken]()
                    taken += 1
            while taken < n_ex:
                extras[taken]()
                taken += 1
            if prev is not None and pv_done < NS + 6:
                while pv_done < NS:
                    emit_pv_slot(prev, pv_done)
                    pv_done += 1
                if pv_done == NS:
                    prev.ots = emit_normalize_muls(prev)
                    pv_done = NS + 2
                elif pv_done == NS + 1:
                    pv_done = NS + 2
                while pv_done < NS + 6:
                    emit_normalize_tp(prev, pv_done - NS - 2)
                    pv_done += 1
            return PrevStage(p, tq, exs)

        # ---- emission ----
        for _rep in range(repeats):
            # startup critical path: pair-0 k weights, first k x-quarter
            # (split in two e-halves so MMs start after half the bytes),
            # then the same for q — only these five DMAs gate the first
            # score slot. Biases/WoS/ident/bv ride the idle ACT queue.
            # k-side criticals on the SP queue, q-side + constants on the
            # Pool queue (SWDGE — its own descriptor generator), so the two
            # startup streams issue in parallel.
            nc.gpsimd.dma_start(out=bqk_sb, in_=bqk_d[:, :])
            nc.sync.dma_start(out=wk2[:, 0:1024], in_=Wk0_d[:, :])
            k0 = proj_thunks(0, kT_d, wk2, kTs[0], bk_sb)
            q0 = proj_thunks(0, qT_d, wq2, qTs[0], bq_sb, eng=nc.gpsimd)
            k0[0]()
            nc.gpsimd.dma_start(out=wq2[:, 0:1024], in_=Wq0_d[:, :])
            q0[0]()
            k0[1](); k0[2](); k0[3]()
            q0[1](); q0[2](); q0[3]()
            # prefetch the stage-0/1 x stream before any bulk weight DMA;
            # kx quarter 1 rides Pool so its transfer follows the criticals
            k0[4]()   # kx quarter 1 (score slot 4, Pool queue)
            k0[8]()   # kx quarter 2 (score slot 8, SP queue)
            wv_tiles = load_wall(WvT_d)
            vpA = vproj_thunks(wv_tiles, 0)
            vpA[0]()  # vx quarter 0 (Pool queue)
            q0[4]()   # qx quarter 1 (stage 1, Pool queue)
            bv_ap = bv_d[:, :]
            bv_bcast_ap = bass.AP(
                tensor=bv_ap.tensor,
                offset=bv_ap.offset,
                ap=[[0, 128], bv_ap.ap[-1]],
            )
            nc.gpsimd.dma_start(out=bv_sb, in_=bv_bcast_ap)
            nc.gpsimd.dma_start(out=ident_sb, in_=ident_d[:, :])

            def late_weights():
                # pairs 1-3 weight walls: first use is stage 3 (~55us);
                # hold them past the startup/stage-0 DMA crunch
                with tc.tile_wait_until(0.028):
                    nc.sync.dma_start(out=wk2[:, 1024:4096], in_=WkR_d[:, :])
                    nc.sync.dma_start(out=wq2[:, 1024:4096], in_=WqR_d[:, :])

            # per-stage extra work, just-in-time:
            #  stage 0: k0 quarters 1-3 (score slots 4/8/12 need them),
            #           q0 quarter 1 (stage 1 start), vpA vst 0-7 + opens
            #  stage 1: vpA vst 8-15 (stage-1 PV consumes all vts by slot
            #           ~12), q0 quarter 2, pairs-1..3 weight walls
            #  stage 2: q0 quarter 3
            extras = {
                0: (k0[5:8] + k0[12:13] + k0[9:12] + vpA[5:6]
                    + k0[13:16] + vpA[1:5] + vpA[10:11]
                    + q0[5:8] + vpA[6:10] + vpA[15:16]),
                1: ([late_weights] + vpA[11:15] + vpA[16:20]
                    + q0[8:9] + q0[9:12]),
                2: q0[12:16],
            }
            # pair p>=1 just-in-time: x-quarter opens a stage ahead of
            # their MMs; k quarters 0/1 + q quarter 0 in stage 4p-1, k
            # quarters 2/3 (slot-8/12 deadlines) + q quarter 1 in stage
            # 4p, q quarters 2/3 in stages 4p+1/4p+2.
            for p in range(1, NP):
                qp = proj_thunks(p, qT_d, wq2, qTs[p], bq_sb)
                kp = proj_thunks(p, kT_d, wk2, kTs[p], bk_sb)
                for sg, th in (
                        (4 * p - 2, kp[0:1] + kp[4:5]),
                        (4 * p - 1, kp[1:4] + kp[5:8] + kp[8:9] + qp[0:1]
                         + kp[12:13] + qp[1:4]),
                        (4 * p, kp[9:12] + kp[13:16] + qp[4:5] + qp[5:8]),
                        (4 * p + 1, qp[8:9] + qp[9:12]),
                        (4 * p + 2, qp[12:13] + qp[13:16])):
                    extras[sg] = extras.get(sg, []) + th
            # second head-quad of V: vst(st) is consumed by stage 9's PV at
            # slot 5+st//2, so the tail quarters can ride the light stages
            # 6 and 9 — keeps every stage's extra-PE load ~3.4us.
            vpB = vproj_thunks(wv_tiles, 1)
            extras[2] = extras.get(2, []) + vpB[0:5]
            extras[3] = extras.get(3, []) + [load_late_inputs]
            extras[5] = extras.get(5, []) + vpB[5:10]
            extras[6] = extras.get(6, []) + vpB[10:15]
            extras[9] = extras.get(9, []) + vpB[15:20]
            # out-proj: pair-3 Onorm for tq0 lands ~slot 13 of stage 13
            # (PE-transpose path), so two op0 units fit at its very end;
            # the rest spread over stages 14/15.
            nops = [lambda: None] * 3
            op0 = outproj_thunks(0)
            extras[13] = extras.get(13, []) + nops + nops + op0[0:2]
            extras[14] = extras.get(14, []) + nops + op0[2:8]
            extras[15] = extras.get(15, []) + nops + outproj_thunks(1)

            # pacing: stage 1's vpA vsts feed stage-1 PV slots (dl=12
            # front-loads them); stage 9's vpB tail must be emitted before
            # the PV slots that read vts (slot 5+st//2); elsewhere even
            # spreading meets the k-quarter slot-8/12 deadlines.
            dls = {1: 12, 4: 10, 8: 10, 9: 7, 12: 10}
            pv_starts = {1: 5}
            prev = None
            for s in range(16):
                p, tq = s // 4, s % 4
                prev = emit_stage(p, tq, prev, extras.get(s, []),
                                  dl=dls.get(s, 16),
                                  pv_start=pv_starts.get(s, 1))

            # tail: PV of the last stage with out-proj(t2) interleaved
            # (its Onorm slices completed at the end of stage 15; hold the
            # last two op2 units to cover the final-normalize window), then
            # per t-tile: normalize on DVE, PE-transpose into a spare psc
            # slot, ACT-copy into Onorm — and out-proj(t3) as narrow units
            # whose PSUM drains ride the otherwise-idle ACT engine, each
            # unit's out-DMA issued as soon as it lands.
            prev.acc = pacc.tile([128, 512], f32, tag="acc", name="acc_ps")
            prev.den = pden.tile([128, 16], f32, tag="den", name="den_ps")
            op2 = outproj_thunks(2)
            for st in range(NS):
                emit_pv_slot(prev, st)
                if st % 2 == 1 and st // 2 < 6:
                    op2[st // 2]()
            rc = small.tile([128, 8], f32, tag="rc", name="rc")
            nc.vector.reciprocal(rc, prev.den[:, 0:8])
            held = op2[6:]
            ots = []
            for k in range(4):
                ot = otp.tile([128, 128], bf16, tag="ot", name="ot")
                for h in range(2):
                    nc.vector.tensor_scalar_mul(
                        ot[:, h * 64:(h + 1) * 64],
                        prev.acc[:, (4 * h + k) * 64:(4 * h + k + 1) * 64],
                        rc[:, 2 * k + h:2 * k + h + 1],
                    )
                ots.append(ot)

            def tail_tp(k):
                tp = psc.tile([128, 128], bf16, tag="sc", name="tp_ps")
                nc.tensor.transpose(tp, ots[k], ident_sb)
                nc.scalar.activation(
                    Onorm[3][:, (12 + k) * 128:(13 + k) * 128], tp, AF.Copy)

            def unit(tt, c):
                op_ps = pmx.tile([128, 512], f32, tag="mx", name="mx_ps")
                for p in range(NP):
                    nc.tensor.matmul(
                        op_ps,
                        Onorm[p][:, tt * 128:(tt + 1) * 128],
                        WoSs[p][:, c * 512:(c + 1) * 512],
                        start=(p == 0),
                        stop=(p == 3),
                    )
                oc = ocp_pool.tile([128, 512], f32, tag="ocp", name="oc")
                nc.scalar.activation(oc, op_ps, AF.Copy)
                eng = nc.sync if c == 0 else nc.scalar
                eng.dma_start(
                    out=out_d[tt * 128:(tt + 1) * 128,
                              c * 512:(c + 1) * 512],
                    in_=oc)

            # transposes run one t-tile ahead of their out-proj units so
            # PE never waits on the ACT Onorm copies
            tail_tp(0); held[0]()
            tail_tp(1); held[1]()
            unit(12, 0); unit(12, 1)
            tail_tp(2)
            unit(13, 0); unit(13, 1)
            tail_tp(3)
            unit(14, 0); unit(14, 1)
            unit(15, 0); unit(15, 1)

    nc.compile()
    return nc


def _get_nc():
    global _cached
    if _cached is None:
        _cached = _build()
    return _cached


def _prep_core_inputs(c, query, key, value, Wq, Wk, Wv, Wo, bq, bk, bv,
                      _cache={}):
    b, g = c // 2, c % 2
    sl = slice(g * DC, (g + 1) * DC)
    key_ = (id(query), b)
    if key_ not in _cache:
        # both cores of a batch share the transposed/cast activations
        _cache.clear()
        _cache[key_] = {
            "qT": query[b].T.astype(_BF16),
            "kT": key[b].T.astype(_BF16),
            "vT": value[b].T.astype(_BF16),
        }
    shared = _cache[key_]

    def pair_major(w):
        # [E, DC] -> per pair p: [128, 8*128] with (p*8+e)*128 indexing
        wt = w[sl].T.astype(_BF16)  # [E, DC]
        blocks = [
            np.ascontiguousarray(
                wt[:, p * 128:(p + 1) * 128]
                .reshape(8, 128, 128).transpose(1, 0, 2).reshape(128, 1024))
            for p in range(NP)
        ]
        return blocks[0], np.ascontiguousarray(np.concatenate(blocks[1:], 1))

    wq0, wqr = pair_major(Wq)
    wk0, wkr = pair_major(Wk)
    bqk = np.concatenate(
        [bq[sl].reshape(NP, 128).T, bk[sl].reshape(NP, 128).T], axis=1)
    return {
        **shared,
        "Wq0": wq0, "WqR": wqr,
        "Wk0": wk0, "WkR": wkr,
        "WvT": Wv[sl].T.astype(_BF16),
        "WoS": Wo[:, sl].T.astype(_BF16),
        "bqk": np.ascontiguousarray(bqk),
        "bv": np.ascontiguousarray(bv[sl].reshape(1, DC)),
        "ident": np.eye(128, dtype=_BF16),
    }


def kernel(**inputs):
    from concourse.bass_utils import run_bass_kernel_spmd

    args = {k: np.asarray(inputs[k], np.float32)
            for k in ("query", "key", "value", "Wq", "Wk", "Wv", "Wo",
                      "bq", "bk", "bv", "bo")}
    _prep_core_inputs.__defaults__[0].clear()
    nc = _get_nc()
    in_maps = [
        _prep_core_inputs(c, args["query"], args["key"], args["value"],
                          args["Wq"], args["Wk"], args["Wv"], args["Wo"],
                          args["bq"], args["bk"], args["bv"])
        for c in range(8)
    ]
    res = run_bass_kernel_spmd(nc, in_maps, core_ids=list(range(8)))
    outs = [r["out"] for r in res.results]
    final = np.empty((B, T, E), np.float32)
    for b in range(B):
        final[b] = outs[2 * b] + outs[2 * b + 1] + args["bo"][None, :]
    return final



# revision 55
# speedup vs baseline: 1.0001x; 1.0001x over previous
"""Multi-head attention (B=4, T=S=2048, E=1024, H=16, D=64) on 8 TRN2 NeuronCores.

Sharding: core c handles batch b=c//2 and head-group g=c%2 (8 of 16 heads).
Each core computes its 8 heads' attention plus the matching column-slice of
the output projection, producing a partial [T, E] f32 output. Host sums the
two partials per batch and adds bo.

On-chip dataflow (all matmuls bf16 with fp32 PSUM accumulation):
  qT[d,t] = WqT.T @ queryT       (d-major projections, per 128-dim head pair)
  kT[d,t] likewise; v[s,d] natural via value.T as the stationary operand
  S.T[s,t] = kT_h.T @ qT_h       (two heads row-packed in the 128-row PE array)
  expS.T   = exp(S.T * 1/8)      (ScalarE, PSUM -> SBUF bf16)
  O[t,d]   = expS.T.T @ v_h      (exp tile stationary, v moving: charges 64
                                  cycles/matmul instead of 512 -> PV at its
                                  cost-model floor; denominators via 1-wide
                                  matmuls against a ones column)
  Onorm    = (O * 1/den).T       (DVE per-partition scalar mul, then an
                                  SBUF->SBUF DMA-transpose back to [d, t])
  partial  = Onorm.T @ WoSlice   (accumulate over the core's 4 head pairs)

Emission is software-pipelined: stage s=(pair, t-quarter); each stage's 16
score-tile slots interleave the previous stage's PV at 2 PV-slots per score
slot (normalize runs mid-stage so the single PV-accumulator PSUM bank is
clear well before reuse) plus spread-out projection / v-projection /
out-projection work, keeping both ScalarE (exp) and PE continuously fed.
"""

from contextlib import ExitStack

import numpy as np
import ml_dtypes

B, T, S, E = 4, 2048, 2048, 1024
H, D = 16, 64
DC = 512          # dims per core (8 heads x 64)
NP = 4            # head pairs per core
NS = S // 128     # 16 s-tiles
NQ = 4            # t-quarters of 512

_BF16 = ml_dtypes.bfloat16

_cached = None


def _build(repeats=1):
    import concourse.bass as bass
    import concourse.mybir as mybir
    import concourse.tile as tile
    from concourse import bacc

    f32 = mybir.dt.float32
    bf16 = mybir.dt.bfloat16
    AF = mybir.ActivationFunctionType

    nc = bacc.Bacc("TRN2", target_bir_lowering=False)

    qT_d = nc.dram_tensor("qT", [E, T], bf16, kind="ExternalInput")
    kT_d = nc.dram_tensor("kT", [E, S], bf16, kind="ExternalInput")
    vT_d = nc.dram_tensor("vT", [E, S], bf16, kind="ExternalInput")
    # q/k projection weights arrive pre-tiled pair-major: Wq0 is pair 0's
    # [128, 8 e-chunks x 128] block (one small contiguous DMA on the
    # startup critical path), WqR the remaining three pairs.
    Wq0_d = nc.dram_tensor("Wq0", [128, 1024], bf16, kind="ExternalInput")
    WqR_d = nc.dram_tensor("WqR", [128, 3072], bf16, kind="ExternalInput")
    Wk0_d = nc.dram_tensor("Wk0", [128, 1024], bf16, kind="ExternalInput")
    WkR_d = nc.dram_tensor("WkR", [128, 3072], bf16, kind="ExternalInput")
    WvT_d = nc.dram_tensor("WvT", [E, DC], bf16, kind="ExternalInput")
    WoS_d = nc.dram_tensor("WoS", [DC, E], bf16, kind="ExternalInput")
    ident_d = nc.dram_tensor("ident", [128, 128], bf16, kind="ExternalInput")
    bqk_d = nc.dram_tensor("bqk", [128, 2 * NP], f32, kind="ExternalInput")
    bv_d = nc.dram_tensor("bv", [1, DC], f32, kind="ExternalInput")
    out_d = nc.dram_tensor("out", [T, E], f32, kind="ExternalOutput")

    with tile.TileContext(nc) as tc, ExitStack() as ctx:
        persist = ctx.enter_context(tc.tile_pool(name="persist", bufs=1))
        psc = ctx.enter_context(tc.tile_pool(name="psc", bufs=2, space="PSUM"))
        pacc = ctx.enter_context(tc.tile_pool(name="pacc", bufs=1, space="PSUM"))
        pden = ctx.enter_context(tc.tile_pool(name="pden", bufs=1, space="PSUM"))
        pmx = ctx.enter_context(tc.tile_pool(name="pmx", bufs=2, space="PSUM"))
        expool = ctx.enter_context(tc.tile_pool(name="expool", bufs=20))
        small = ctx.enter_context(tc.tile_pool(name="small", bufs=10))
        otp = ctx.enter_context(tc.tile_pool(name="otp", bufs=8))
        ocp_pool = ctx.enter_context(tc.tile_pool(name="ocp", bufs=3))
        xin = ctx.enter_context(tc.tile_pool(name="xin", bufs=7))
        wpool = ctx.enter_context(tc.tile_pool(name="wts", bufs=1))

        # ---- persistent SBUF tiles ----
        qTs = [persist.tile([128, T], bf16, tag=f"qT{p}", name=f"qT{p}") for p in range(NP)]
        kTs = [persist.tile([128, S], bf16, tag=f"kT{p}", name=f"kT{p}") for p in range(NP)]
        vts = [persist.tile([128, DC], bf16, tag=f"v{st}", name=f"v{st}") for st in range(NS)]
        WoSs = [persist.tile([128, E], bf16, tag=f"wo{p}", name=f"wo{p}") for p in range(NP)]
        Onorm = [persist.tile([128, T], bf16, tag=f"on{p}", name=f"on{p}") for p in range(NP)]
        bqk_sb = persist.tile([128, 2 * NP], f32, tag="bqk", name="bqk_sb")
        bq_sb = bqk_sb[:, 0:NP]
        bk_sb = bqk_sb[:, NP:2 * NP]
        bv_sb = persist.tile([128, DC], f32, tag="bv", name="bv_sb")
        ones_sb = persist.tile([128, 1], bf16, tag="ones", name="ones_sb")
        ident_sb = persist.tile([128, 128], bf16, tag="ident", name="ident_sb")
        # pair-major q/k weight walls: slice (p, e) at cols (p*8+e)*128
        wq2 = persist.tile([128, 4096], bf16, tag="wq2", name="wq2")
        wk2 = persist.tile([128, 4096], bf16, tag="wk2", name="wk2")

        nc.vector.memset(ones_sb, 1.0)

        def load_late_inputs():
            """Output-projection weights: first use ~stage 13; pin them
            past the projection-heavy first third of the timeline."""
            with tc.tile_wait_until(0.065):
                for p in range(NP):
                    nc.scalar.dma_start(out=WoSs[p],
                                        in_=WoS_d[p * 128:(p + 1) * 128, :])

        def load_wall(dram):
            """All 8 e-chunks of one weight set, as two strided DMAs:
            wall[:, e*DC + c] = dram[e*128 + p, c]."""
            t_ = wpool.tile([128, 8 * DC], bf16, tag="w", name="wall")
            for g in range(2):
                nc.sync.dma_start(
                    out=t_[:, g * 4 * DC:(g + 1) * 4 * DC
                           ].rearrange("p (e c) -> p e c", c=DC),
                    in_=dram[g * 512:(g + 1) * 512, :
                             ].rearrange("(e p) c -> p e c", p=128))
            return t_

        def proj_thunks(p, x_dram, wall, dst, bias_sb, eng=None):
            """One pair's q/k projection, quarter-granular: per t-quarter
            one xin tile carrying all 8 e-chunks ([128, 8x512]) loaded as
            two e-half DMAs, then 8 accumulating MMs in a pmx tile and a
            bias-add drain. Thunk layout: [open, mm03, mm47, drain] x 4
            quarters. eng picks the DMA-issue queue per quarter (SP
            default; Pool gives startup-critical loads their own DGE)."""
            engs = eng if isinstance(eng, list) else [eng or nc.sync] * 4
            thunks = []
            for q in range(4):
                xq = []

                def open_q(q=q, xq=xq, qeng=engs[q]):
                    # two e-half DMAs: halves the DMA_ENGINES blocking
                    # granularity (transposes/outputs queue behind these)
                    xt = xin.tile([128, 4096], bf16, tag="xin", name="xin")
                    v = xt.rearrange("p (e t) -> p e t", e=8)
                    for g in range(2):
                        qeng.dma_start(
                            out=v[:, g * 4:(g + 1) * 4, :],
                            in_=x_dram[g * 512:(g + 1) * 512,
                                       q * 512:(q + 1) * 512
                                       ].rearrange("(e p) t -> p e t",
                                                   p=128))
                    xq.append(xt)

                thunks.append(open_q)
                ps = []

                def mme(lo, hi, ps=ps, xq=xq):
                    if lo == 0:
                        ps.append(pmx.tile([128, 512], f32, tag="mx",
                                           name="mx_ps"))
                    for e in range(lo, hi):
                        nc.tensor.matmul(
                            ps[0],
                            wall[:, (p * 8 + e) * 128:(p * 8 + e + 1) * 128],
                            xq[0][:, e * 512:(e + 1) * 512],
                            start=(e == 0),
                            stop=(e == 7),
                        )

                thunks.append(lambda f=mme: f(0, 4))
                thunks.append(lambda f=mme: f(4, 8))

                def close_q(q=q, ps=ps, xq=xq):
                    nc.vector.tensor_scalar_add(
                        dst[:, q * 512:(q + 1) * 512],
                        ps[0], bias_sb[:, p:p + 1])
                    ps.clear()
                    xq.clear()

                thunks.append(close_q)
            return thunks

        def vproj_thunks(wv_tiles, dh):
            """V projection for head-quad dh (4 heads, N=256), streamed in
            four s-quarters: per quarter one xin DMA + 4 s-tile MM groups.
            dh=0 feeds pairs 0-1 (needed by stage 1); dh=1 feeds pairs 2-3
            (needed from stage 9). Thunks: [open, vst x4] x 4 quarters."""
            thunks = []
            for q in range(4):
                vq = []

                def open_q(q=q, vq=vq):
                    # v loads ride the idle Pool engine's SWDGE path: no
                    # HWDGE contention, keeps the SP sequencer free
                    vt = xin.tile([128, 4096], bf16, tag="xin", name="vxin")
                    v = vt.rearrange("p (e t) -> p e t", e=8)
                    for g in range(2):
                        nc.gpsimd.dma_start(
                            out=v[:, g * 4:(g + 1) * 4, :],
                            in_=vT_d[g * 512:(g + 1) * 512,
                                     q * 512:(q + 1) * 512
                                     ].rearrange("(e p) t -> p e t", p=128))
                    vq.append(vt)

                thunks.append(open_q)
                for sti in range(4):
                    def vst(sti=sti, q=q, vq=vq):
                        st = q * 4 + sti
                        ps = pmx.tile([128, 512], f32, tag="mx", name="mx_ps")
                        for e in range(8):
                            nc.tensor.matmul(
                                ps[:, 0:256],
                                vq[0][:, e * 512 + sti * 128:
                                      e * 512 + (sti + 1) * 128],
                                wv_tiles[:, e * DC + dh * 256:
                                         e * DC + (dh + 1) * 256],
                                start=(e == 0),
                                stop=(e == 7),
                            )
                        nc.vector.tensor_add(
                            vts[st][:, dh * 256:(dh + 1) * 256],
                            ps[:, 0:256],
                            bv_sb[:, dh * 256:(dh + 1) * 256],
                        )
                        if sti == 3:
                            vq.clear()
                    thunks.append(vst)
            return thunks

        def outproj_thunks(tq):
            thunks = []
            for tt in range(tq * 4, tq * 4 + 4):
                for c in range(2):
                    def unit(tt=tt, c=c):
                        op_ps = pmx.tile([128, 512], f32, tag="mx", name="mx_ps")
                        for p in range(NP):
                            nc.tensor.matmul(
                                op_ps,
                                Onorm[p][:, tt * 128:(tt + 1) * 128],
                                WoSs[p][:, c * 512:(c + 1) * 512],
                                start=(p == 0),
                                stop=(p == 3),
                            )
                        oc = ocp_pool.tile([128, 512], f32, tag="ocp", name="oc")
                        nc.vector.tensor_copy(oc, op_ps)
                        nc.sync.dma_start(
                            out=out_d[tt * 128:(tt + 1) * 128,
                                      c * 512:(c + 1) * 512],
                            in_=oc)
                    thunks.append(unit)
            return thunks

        class PrevStage:
            def __init__(self, p, tq, exs):
                self.p, self.tq, self.exs = p, tq, exs
                self.acc = None   # [128 t, 512]: 8 x 64-wide accums, idx 4h+k
                self.den = None   # [128 t, 16]: cols 2k+h
                self.ots = None   # normalized [t, 128] tiles awaiting transpose

        def emit_pv_slot(prev, st):
            """PV for one s-tile of the previous stage: per (head h, t-tile
            k), a 64-wide main matmul (exp stationary, v moving) plus a
            1-wide denominator matmul against the ones column. One PSUM
            accumulation group per bank (lazy per-byte init)."""
            for h in range(2):
                vsl = vts[st][:, (2 * prev.p + h) * 64:(2 * prev.p + h + 1) * 64]
                for k in range(4):
                    idx = 4 * h + k
                    exsl = prev.exs[st][:, h * 512 + k * 128:
                                        h * 512 + (k + 1) * 128]
                    first = (st == 0 and idx == 0)
                    last = (st == NS - 1 and idx == 7)
                    nc.tensor.matmul(
                        prev.acc[:, idx * 64:(idx + 1) * 64],
                        exsl, vsl,
                        start=first, stop=last,
                    )
                    nc.tensor.matmul(
                        prev.den[:, 2 * k + h:2 * k + h + 1],
                        exsl, ones_sb,
                        start=first, stop=last,
                    )

        def emit_normalize_muls(prev):
            """Phase A of normalizing the previous stage's PV accumulators:
            one reciprocal over all 8 denominators, then per t-tile k a
            scalar-multiply of both heads' 64 cols into a [t, 128] bf16
            tile (DVE only — PE keeps streaming)."""
            rc = small.tile([128, 8], f32, tag="rc", name="rc")
            nc.vector.reciprocal(rc, prev.den[:, 0:8])
            ots = []
            for k in range(4):
                ot = otp.tile([128, 128], bf16, tag="ot", name="ot")
                for h in range(2):
                    nc.vector.tensor_scalar_mul(
                        ot[:, h * 64:(h + 1) * 64],
                        prev.acc[:, (4 * h + k) * 64:(4 * h + k + 1) * 64],
                        rc[:, 2 * k + h:2 * k + h + 1],
                    )
                ots.append(ot)
            return ots

        def emit_normalize_tp(prev, k):
            """Phase B (spread one per slot so the pmx ring and DVE copies
            never block PE): PE row-column transpose of one [t, 128] tile
            through a pmx slot, DVE copy into the [d, t]-major Onorm
            slice. Keeps normalize off the DMA queues entirely."""
            t0 = prev.tq * 4
            # ride the pacc ring: the acc tile was already consumed by the
            # muls, and the next stage's acc alloc waits for tp3's copy —
            # keeps transposes out of the busy pmx ring
            tp = pacc.tile([128, 128], bf16, tag="acc", name="tp_ps")
            nc.tensor.transpose(tp, prev.ots[k], ident_sb)
            nc.vector.tensor_copy(
                Onorm[prev.p][:, (t0 + k) * 128:(t0 + k + 1) * 128], tp)

        def emit_stage(p, tq, prev, extras, dl=6, pv_start=0):
            """16 score slots for (p, tq); interleave prev stage's PV at two
            PV-slots per score slot from `pv_start` (all its exp tiles
            already exist), normalizing mid-stage so the single PV
            accumulator bank clears well before the next stage reuses it;
            extra thunks all emitted by slot `dl`. Returns this stage's
            PrevStage record."""
            t0 = tq * 512
            exs = []
            if prev is not None:
                prev.acc = pacc.tile([128, 512], f32, tag="acc", name="acc_ps")
                prev.den = pden.tile([128, 16], f32, tag="den", name="den_ps")
            n_ex = len(extras)
            taken = 0
            pv_done = 0
            for st in range(NS):
                sc_ps = psc.tile([128, 1024], f32, tag="sc", name="sc_ps")
                nc.tensor.matmul(
                    sc_ps[:, 0:512],
                    kTs[p][0:64, st * 128:(st + 1) * 128],
                    qTs[p][0:64, t0:t0 + 512],
                    start=True, stop=True,
                    tile_position=(0, 0),
                )
                nc.tensor.matmul(
                    sc_ps[:, 512:1024],
                    kTs[p][64:128, st * 128:(st + 1) * 128],
                    qTs[p][64:128, t0:t0 + 512],
                    start=True, stop=True,
                    tile_position=(64, 0),
                )
                ex = expool.tile([128, 1024], bf16, tag="ex", name="ex")
                nc.scalar.activation(ex, sc_ps, AF.Exp, scale=0.125)
                exs.append(ex)
                if prev is not None and st >= pv_start:
                    if NS + 1 < pv_done < NS + 6:
                        emit_normalize_tp(prev, pv_done - NS - 2)
                        pv_done += 1
                    elif pv_done == NS + 1:
                        pv_done += 1   # one settle slot for the DVE muls
                    for _ in range(2):
                        if pv_done < NS:
                            emit_pv_slot(prev, pv_done)
                            pv_done += 1
                    if pv_done == NS:
                        prev.ots = emit_normalize_muls(prev)
                        pv_done = NS + 1
                want = (n_ex * min(st + 1, dl) + dl - 1) // dl
                while taken < want:
                    extras[taken]()
                    taken += 1
            while taken < n_ex:
                extras[taken]()
                taken += 1
            if prev is not None and pv_done < NS + 6:
                while pv_done < NS:
                    emit_pv_slot(prev, pv_done)
                    pv_done += 1
                if pv_done == NS:
                    prev.ots = emit_normalize_muls(prev)
                    pv_done = NS + 2
                elif pv_done == NS + 1:
                    pv_done = NS + 2
                while pv_done < NS + 6:
                    emit_normalize_tp(prev, pv_done - NS - 2)
                    pv_done += 1
            return PrevStage(p, tq, exs)

        # ---- emission ----
        for _rep in range(repeats):
            # startup critical path: pair-0 k weights, first k x-quarter
            # (split in two e-halves so MMs start after half the bytes),
            # then the same for q — only these five DMAs gate the first
            # score slot. Biases/WoS/ident/bv ride the idle ACT queue.
            # k-side criticals on the SP queue, q-side + constants on the
            # Pool queue (SWDGE — its own descriptor generator), so the two
            # startup streams issue in parallel.
            nc.gpsimd.dma_start(out=bqk_sb, in_=bqk_d[:, :])
            nc.sync.dma_start(out=wk2[:, 0:1024], in_=Wk0_d[:, :])
            k0 = proj_thunks(0, kT_d, wk2, kTs[0], bk_sb)
            q0 = proj_thunks(0, qT_d, wq2, qTs[0], bq_sb, eng=nc.gpsimd)
            k0[0]()
            nc.gpsimd.dma_start(out=wq2[:, 0:1024], in_=Wq0_d[:, :])
            q0[0]()
            k0[1](); k0[2](); k0[3]()
            q0[1](); q0[2](); q0[3]()
            # prefetch the stage-0/1 x stream before any bulk weight DMA;
            # kx quarter 1 rides Pool so its transfer follows the criticals
            k0[4]()   # kx quarter 1 (score slot 4, Pool queue)
            k0[8]()   # kx quarter 2 (score slot 8, SP queue)
            wv_tiles = load_wall(WvT_d)
            vpA = vproj_thunks(wv_tiles, 0)
            vpA[0]()  # vx quarter 0 (Pool queue)
            q0[4]()   # qx quarter 1 (stage 1, Pool queue)
            bv_ap = bv_d[:, :]
            bv_bcast_ap = bass.AP(
                tensor=bv_ap.tensor,
                offset=bv_ap.offset,
                ap=[[0, 128], bv_ap.ap[-1]],
            )
            nc.gpsimd.dma_start(out=bv_sb, in_=bv_bcast_ap)
            nc.gpsimd.dma_start(out=ident_sb, in_=ident_d[:, :])

            def late_weights():
                # pairs 1-3 weight walls: first use is stage 3 (~55us);
                # hold them past the startup/stage-0 DMA crunch
                with tc.tile_wait_until(0.028):
                    nc.sync.dma_start(out=wk2[:, 1024:4096], in_=WkR_d[:, :])
                    nc.sync.dma_start(out=wq2[:, 1024:4096], in_=WqR_d[:, :])

            # per-stage extra work, just-in-time:
            #  stage 0: k0 quarters 1-3 (score slots 4/8/12 need them),
            #           q0 quarter 1 (stage 1 start), vpA vst 0-7 + opens
            #  stage 1: vpA vst 8-15 (stage-1 PV consumes all vts by slot
            #           ~12), q0 quarter 2, pairs-1..3 weight walls
            #  stage 2: q0 quarter 3
            extras = {
                0: (k0[5:8] + k0[12:13] + k0[9:12] + vpA[5:6]
                    + k0[13:16] + vpA[1:5] + vpA[10:11]
                    + q0[5:8] + vpA[6:10] + vpA[15:16]),
                1: ([late_weights] + vpA[11:15] + vpA[16:20]
                    + q0[8:9] + q0[9:12]),
                2: q0[12:16],
            }
            # pair p>=1 just-in-time: x-quarter opens a stage ahead of
            # their MMs; k quarters 0/1 + q quarter 0 in stage 4p-1, k
            # quarters 2/3 (slot-8/12 deadlines) + q quarter 1 in stage
            # 4p, q quarters 2/3 in stages 4p+1/4p+2.
            for p in range(1, NP):
                qp = proj_thunks(p, qT_d, wq2, qTs[p], bq_sb)
                kp = proj_thunks(p, kT_d, wk2, kTs[p], bk_sb)
                for sg, th in (
                        (4 * p - 2, kp[0:1] + kp[4:5]),
                        (4 * p - 1, kp[1:4] + kp[5:8] + kp[8:9] + qp[0:1]
                         + kp[12:13] + qp[1:4]),
                        (4 * p, kp[9:12] + kp[13:16] + qp[4:5] + qp[5:8]),
                        (4 * p + 1, qp[8:9] + qp[9:12]),
                        (4 * p + 2, qp[12:13] + qp[13:16])):
                    extras[sg] = extras.get(sg, []) + th
            # second head-quad of V: vst(st) is consumed by stage 9's PV at
            # slot 5+st//2, so the tail quarters can ride the light stages
            # 6 and 9 — keeps every stage's extra-PE load ~3.4us.
            vpB = vproj_thunks(wv_tiles, 1)
            extras[2] = extras.get(2, []) + vpB[0:5]
            extras[3] = extras.get(3, []) + [load_late_inputs]
            extras[5] = extras.get(5, []) + vpB[5:10]
            extras[6] = extras.get(6, []) + vpB[10:15]
            extras[9] = extras.get(9, []) + vpB[15:20]
            # out-proj: pair-3 Onorm for tq0 lands ~slot 13 of stage 13
            # (PE-transpose path), so two op0 units fit at its very end;
            # the rest spread over stages 14/15.
            nops = [lambda: None] * 3
            op0 = outproj_thunks(0)
            extras[13] = extras.get(13, []) + nops + nops + op0[0:2]
            extras[14] = extras.get(14, []) + nops + op0[2:8]
            extras[15] = extras.get(15, []) + nops + outproj_thunks(1)

            # pacing: stage 1's vpA vsts feed stage-1 PV slots (dl=12
            # front-loads them); stage 9's vpB tail must be emitted before
            # the PV slots that read vts (slot 5+st//2); elsewhere even
            # spreading meets the k-quarter slot-8/12 deadlines.
            dls = {1: 12, 4: 10, 8: 10, 9: 7, 12: 10}
            pv_starts = {1: 5}
            prev = None
            for s in range(16):
                p, tq = s // 4, s % 4
                prev = emit_stage(p, tq, prev, extras.get(s, []),
                                  dl=dls.get(s, 16),
                                  pv_start=pv_starts.get(s, 1))

            # tail: PV of the last stage with out-proj(t2) interleaved
            # (its Onorm slices completed at the end of stage 15; hold the
            # last two op2 units to cover the final-normalize window), then
            # per t-tile: normalize on DVE, PE-transpose into a spare psc
            # slot, ACT-copy into Onorm — and out-proj(t3) as narrow units
            # whose PSUM drains ride the otherwise-idle ACT engine, each
            # unit's out-DMA issued as soon as it lands.
            prev.acc = pacc.tile([128, 512], f32, tag="acc", name="acc_ps")
            prev.den = pden.tile([128, 16], f32, tag="den", name="den_ps")
            op2 = outproj_thunks(2)
            for st in range(NS):
                emit_pv_slot(prev, st)
                if st % 2 == 1 and st // 2 < 6:
                    op2[st // 2]()
            rc = small.tile([128, 8], f32, tag="rc", name="rc")
            nc.vector.reciprocal(rc, prev.den[:, 0:8])
            held = op2[6:]
            ots = []
            for k in range(4):
                ot = otp.tile([128, 128], bf16, tag="ot", name="ot")
                for h in range(2):
                    nc.vector.tensor_scalar_mul(
                        ot[:, h * 64:(h + 1) * 64],
                        prev.acc[:, (4 * h + k) * 64:(4 * h + k + 1) * 64],
                        rc[:, 2 * k + h:2 * k + h + 1],
                    )
                ots.append(ot)

            def tail_tp(k):
                tp = psc.tile([128, 128], bf16, tag="sc", name="tp_ps")
                nc.tensor.transpose(tp, ots[k], ident_sb)
                nc.scalar.activation(
                    Onorm[3][:, (12 + k) * 128:(13 + k) * 128], tp, AF.Copy)

            def unit(tt, c):
                op_ps = pmx.tile([128, 512], f32, tag="mx", name="mx_ps")
                for p in range(NP):
                    nc.tensor.matmul(
                        op_ps,
                        Onorm[p][:, tt * 128:(tt + 1) * 128],
                        WoSs[p][:, c * 512:(c + 1) * 512],
                        start=(p == 0),
                        stop=(p == 3),
                    )
                oc = ocp_pool.tile([128, 512], f32, tag="ocp", name="oc")
                nc.scalar.activation(oc, op_ps, AF.Copy)
                eng = nc.sync if c == 0 else nc.scalar
                eng.dma_start(
                    out=out_d[tt * 128:(tt + 1) * 128,
                              c * 512:(c + 1) * 512],
                    in_=oc)

            # transposes run one t-tile ahead of their out-proj units so
            # PE never waits on the ACT Onorm copies
            tail_tp(0); held[0]()
            tail_tp(1); held[1]()
            unit(12, 0); unit(12, 1)
            tail_tp(2)
            unit(13, 0); unit(13, 1)
            tail_tp(3)
            unit(14, 0); unit(14, 1)
            unit(15, 0); unit(15, 1)

    nc.compile()
    return nc


def _get_nc():
    global _cached
    if _cached is None:
        _cached = _build()
    return _cached


def _prep_core_inputs(c, query, key, value, Wq, Wk, Wv, Wo, bq, bk, bv,
                      _cache={}):
    b, g = c // 2, c % 2
    sl = slice(g * DC, (g + 1) * DC)
    key_ = (id(query), b)
    if key_ not in _cache:
        # both cores of a batch share the transposed/cast activations
        _cache.clear()
        _cache[key_] = {
            "qT": query[b].T.astype(_BF16),
            "kT": key[b].T.astype(_BF16),
            "vT": value[b].T.astype(_BF16),
        }
    shared = _cache[key_]

    def pair_major(w):
        # [E, DC] -> per pair p: [128, 8*128] with (p*8+e)*128 indexing
        wt = w[sl].T.astype(_BF16)  # [E, DC]
        blocks = [
            np.ascontiguousarray(
                wt[:, p * 128:(p + 1) * 128]
                .reshape(8, 128, 128).transpose(1, 0, 2).reshape(128, 1024))
            for p in range(NP)
        ]
        return blocks[0], np.ascontiguousarray(np.concatenate(blocks[1:], 1))

    wq0, wqr = pair_major(Wq)
    wk0, wkr = pair_major(Wk)
    bqk = np.concatenate(
        [bq[sl].reshape(NP, 128).T, bk[sl].reshape(NP, 128).T], axis=1)
    return {
        **shared,
        "Wq0": wq0, "WqR": wqr,
        "Wk0": wk0, "WkR": wkr,
        "WvT": Wv[sl].T.astype(_BF16),
        "WoS": Wo[:, sl].T.astype(_BF16),
        "bqk": np.ascontiguousarray(bqk),
        "bv": np.ascontiguousarray(bv[sl].reshape(1, DC)),
        "ident": np.eye(128, dtype=_BF16),
    }


def kernel(**inputs):
    from concourse.bass_utils import run_bass_kernel_spmd

    args = {k: np.asarray(inputs[k], np.float32)
            for k in ("query", "key", "value", "Wq", "Wk", "Wv", "Wo",
                      "bq", "bk", "bv", "bo")}
    _prep_core_inputs.__defaults__[0].clear()
    nc = _get_nc()
    in_maps = [
        _prep_core_inputs(c, args["query"], args["key"], args["value"],
                          args["Wq"], args["Wk"], args["Wv"], args["Wo"],
                          args["bq"], args["bk"], args["bv"])
        for c in range(8)
    ]
    res = run_bass_kernel_spmd(nc, in_maps, core_ids=list(range(8)))
    outs = [r["out"] for r in res.results]
    final = np.empty((B, T, E), np.float32)
    for b in range(B):
        final[b] = outs[2 * b] + outs[2 * b + 1] + args["bo"][None, :]
    return final



# revision 56
# speedup vs baseline: 1.0072x; 1.0071x over previous
"""Multi-head attention (B=4, T=S=2048, E=1024, H=16, D=64) on 8 TRN2 NeuronCores.

Sharding: core c handles batch b=c//2 and head-group g=c%2 (8 of 16 heads).
Each core computes its 8 heads' attention plus the matching column-slice of
the output projection, producing a partial [T, E] f32 output. Host sums the
two partials per batch and adds bo.

On-chip dataflow (all matmuls bf16 with fp32 PSUM accumulation):
  qT[d,t] = WqT.T @ queryT       (d-major projections, per 128-dim head pair)
  kT[d,t] likewise; v[s,d] natural via value.T as the stationary operand
  S.T[s,t] = kT_h.T @ qT_h       (two heads row-packed in the 128-row PE array)
  expS.T   = exp(S.T * 1/8)      (ScalarE, PSUM -> SBUF bf16)
  O[t,d]   = expS.T.T @ v_h      (exp tile stationary, v moving: charges 64
                                  cycles/matmul instead of 512 -> PV at its
                                  cost-model floor; denominators via 1-wide
                                  matmuls against a ones column)
  Onorm    = (O * 1/den).T       (DVE per-partition scalar mul, then an
                                  SBUF->SBUF DMA-transpose back to [d, t])
  partial  = Onorm.T @ WoSlice   (accumulate over the core's 4 head pairs)

Emission is software-pipelined: stage s=(pair, t-quarter); each stage's 16
score-tile slots interleave the previous stage's PV at 2 PV-slots per score
slot (normalize runs mid-stage so the single PV-accumulator PSUM bank is
clear well before reuse) plus spread-out projection / v-projection /
out-projection work, keeping both ScalarE (exp) and PE continuously fed.
"""

from contextlib import ExitStack

import numpy as np
import ml_dtypes

B, T, S, E = 4, 2048, 2048, 1024
H, D = 16, 64
DC = 512          # dims per core (8 heads x 64)
NP = 4            # head pairs per core
NS = S // 128     # 16 s-tiles
NQ = 4            # t-quarters of 512

_BF16 = ml_dtypes.bfloat16

_cached = None


def _build(repeats=1):
    import concourse.bass as bass
    import concourse.mybir as mybir
    import concourse.tile as tile
    from concourse import bacc

    f32 = mybir.dt.float32
    bf16 = mybir.dt.bfloat16
    AF = mybir.ActivationFunctionType

    nc = bacc.Bacc("TRN2", target_bir_lowering=False)

    qT_d = nc.dram_tensor("qT", [E, T], bf16, kind="ExternalInput")
    kT_d = nc.dram_tensor("kT", [E, S], bf16, kind="ExternalInput")
    vT_d = nc.dram_tensor("vT", [E, S], bf16, kind="ExternalInput")
    # q/k projection weights arrive pre-tiled pair-major: Wq0 is pair 0's
    # [128, 8 e-chunks x 128] block (one small contiguous DMA on the
    # startup critical path), WqR the remaining three pairs.
    Wq0_d = nc.dram_tensor("Wq0", [128, 1024], bf16, kind="ExternalInput")
    WqR_d = nc.dram_tensor("WqR", [128, 3072], bf16, kind="ExternalInput")
    Wk0_d = nc.dram_tensor("Wk0", [128, 1024], bf16, kind="ExternalInput")
    WkR_d = nc.dram_tensor("WkR", [128, 3072], bf16, kind="ExternalInput")
    WvT_d = nc.dram_tensor("WvT", [E, DC], bf16, kind="ExternalInput")
    WoS_d = nc.dram_tensor("WoS", [DC, E], bf16, kind="ExternalInput")
    ident_d = nc.dram_tensor("ident", [128, 128], bf16, kind="ExternalInput")
    bqk_d = nc.dram_tensor("bqk", [128, 2 * NP], f32, kind="ExternalInput")
    bv_d = nc.dram_tensor("bv", [1, DC], f32, kind="ExternalInput")
    out_d = nc.dram_tensor("out", [T, E], f32, kind="ExternalOutput")

    with tile.TileContext(nc) as tc, ExitStack() as ctx:
        persist = ctx.enter_context(tc.tile_pool(name="persist", bufs=1))
        psc = ctx.enter_context(tc.tile_pool(name="psc", bufs=2, space="PSUM"))
        pacc = ctx.enter_context(tc.tile_pool(name="pacc", bufs=1, space="PSUM"))
        pden = ctx.enter_context(tc.tile_pool(name="pden", bufs=1, space="PSUM"))
        pmx = ctx.enter_context(tc.tile_pool(name="pmx", bufs=2, space="PSUM"))
        expool = ctx.enter_context(tc.tile_pool(name="expool", bufs=20))
        small = ctx.enter_context(tc.tile_pool(name="small", bufs=10))
        otp = ctx.enter_context(tc.tile_pool(name="otp", bufs=12))
        ocp_pool = ctx.enter_context(tc.tile_pool(name="ocp", bufs=4))
        xin = ctx.enter_context(tc.tile_pool(name="xin", bufs=7))
        wpool = ctx.enter_context(tc.tile_pool(name="wts", bufs=1))

        # ---- persistent SBUF tiles ----
        qTs = [persist.tile([128, T], bf16, tag=f"qT{p}", name=f"qT{p}") for p in range(NP)]
        kTs = [persist.tile([128, S], bf16, tag=f"kT{p}", name=f"kT{p}") for p in range(NP)]
        vts = [persist.tile([128, DC], bf16, tag=f"v{st}", name=f"v{st}") for st in range(NS)]
        WoSs = [persist.tile([128, E], bf16, tag=f"wo{p}", name=f"wo{p}") for p in range(NP)]
        Onorm = [persist.tile([128, T], bf16, tag=f"on{p}", name=f"on{p}") for p in range(NP)]
        bqk_sb = persist.tile([128, 2 * NP], f32, tag="bqk", name="bqk_sb")
        bq_sb = bqk_sb[:, 0:NP]
        bk_sb = bqk_sb[:, NP:2 * NP]
        bv_sb = persist.tile([128, DC], f32, tag="bv", name="bv_sb")
        ones_sb = persist.tile([128, 1], bf16, tag="ones", name="ones_sb")
        ident_sb = persist.tile([128, 128], bf16, tag="ident", name="ident_sb")
        # pair-major q/k weight walls: slice (p, e) at cols (p*8+e)*128
        wq2 = persist.tile([128, 4096], bf16, tag="wq2", name="wq2")
        wk2 = persist.tile([128, 4096], bf16, tag="wk2", name="wk2")

        nc.vector.memset(ones_sb, 1.0)

        def load_late_inputs():
            """Output-projection weights: first use ~stage 13; pin them
            past the projection-heavy first third of the timeline."""
            with tc.tile_wait_until(0.065):
                for p in range(NP):
                    nc.scalar.dma_start(out=WoSs[p],
                                        in_=WoS_d[p * 128:(p + 1) * 128, :])

        def load_wall(dram):
            """All 8 e-chunks of one weight set, as two strided DMAs:
            wall[:, e*DC + c] = dram[e*128 + p, c]."""
            t_ = wpool.tile([128, 8 * DC], bf16, tag="w", name="wall")
            for g in range(2):
                nc.sync.dma_start(
                    out=t_[:, g * 4 * DC:(g + 1) * 4 * DC
                           ].rearrange("p (e c) -> p e c", c=DC),
                    in_=dram[g * 512:(g + 1) * 512, :
                             ].rearrange("(e p) c -> p e c", p=128))
            return t_

        def proj_thunks(p, x_dram, wall, dst, bias_sb, eng=None):
            """One pair's q/k projection, quarter-granular: per t-quarter
            one xin tile carrying all 8 e-chunks ([128, 8x512]) loaded as
            two e-half DMAs, then 8 accumulating MMs in a pmx tile and a
            bias-add drain. Thunk layout: [open, mm03, mm47, drain] x 4
            quarters. eng picks the DMA-issue queue per quarter (SP
            default; Pool gives startup-critical loads their own DGE)."""
            engs = eng if isinstance(eng, list) else [eng or nc.sync] * 4
            thunks = []
            for q in range(4):
                xq = []

                def open_q(q=q, xq=xq, qeng=engs[q]):
                    # two e-half DMAs: halves the DMA_ENGINES blocking
                    # granularity (transposes/outputs queue behind these)
                    xt = xin.tile([128, 4096], bf16, tag="xin", name="xin")
                    v = xt.rearrange("p (e t) -> p e t", e=8)
                    for g in range(2):
                        qeng.dma_start(
                            out=v[:, g * 4:(g + 1) * 4, :],
                            in_=x_dram[g * 512:(g + 1) * 512,
                                       q * 512:(q + 1) * 512
                                       ].rearrange("(e p) t -> p e t",
                                                   p=128))
                    xq.append(xt)

                thunks.append(open_q)
                ps = []

                def mme(lo, hi, ps=ps, xq=xq):
                    if lo == 0:
                        ps.append(pmx.tile([128, 512], f32, tag="mx",
                                           name="mx_ps"))
                    for e in range(lo, hi):
                        nc.tensor.matmul(
                            ps[0],
                            wall[:, (p * 8 + e) * 128:(p * 8 + e + 1) * 128],
                            xq[0][:, e * 512:(e + 1) * 512],
                            start=(e == 0),
                            stop=(e == 7),
                        )

                thunks.append(lambda f=mme: f(0, 4))
                thunks.append(lambda f=mme: f(4, 8))

                def close_q(q=q, ps=ps, xq=xq):
                    nc.vector.tensor_scalar_add(
                        dst[:, q * 512:(q + 1) * 512],
                        ps[0], bias_sb[:, p:p + 1])
                    ps.clear()
                    xq.clear()

                thunks.append(close_q)
            return thunks

        def vproj_thunks(wv_tiles, dh):
            """V projection for head-quad dh (4 heads, N=256), streamed in
            four s-quarters: per quarter one xin DMA + 4 s-tile MM groups.
            dh=0 feeds pairs 0-1 (needed by stage 1); dh=1 feeds pairs 2-3
            (needed from stage 9). Thunks: [open, vst x4] x 4 quarters."""
            thunks = []
            for q in range(4):
                vq = []

                def open_q(q=q, vq=vq):
                    # v loads ride the idle Pool engine's SWDGE path: no
                    # HWDGE contention, keeps the SP sequencer free
                    vt = xin.tile([128, 4096], bf16, tag="xin", name="vxin")
                    v = vt.rearrange("p (e t) -> p e t", e=8)
                    for g in range(2):
                        nc.gpsimd.dma_start(
                            out=v[:, g * 4:(g + 1) * 4, :],
                            in_=vT_d[g * 512:(g + 1) * 512,
                                     q * 512:(q + 1) * 512
                                     ].rearrange("(e p) t -> p e t", p=128))
                    vq.append(vt)

                thunks.append(open_q)
                for sti in range(4):
                    def vst(sti=sti, q=q, vq=vq):
                        st = q * 4 + sti
                        ps = pmx.tile([128, 512], f32, tag="mx", name="mx_ps")
                        for e in range(8):
                            nc.tensor.matmul(
                                ps[:, 0:256],
                                vq[0][:, e * 512 + sti * 128:
                                      e * 512 + (sti + 1) * 128],
                                wv_tiles[:, e * DC + dh * 256:
                                         e * DC + (dh + 1) * 256],
                                start=(e == 0),
                                stop=(e == 7),
                            )
                        nc.vector.tensor_add(
                            vts[st][:, dh * 256:(dh + 1) * 256],
                            ps[:, 0:256],
                            bv_sb[:, dh * 256:(dh + 1) * 256],
                        )
                        if sti == 3:
                            vq.clear()
                    thunks.append(vst)
            return thunks

        def outproj_thunks(tq):
            thunks = []
            for tt in range(tq * 4, tq * 4 + 4):
                for c in range(2):
                    def unit(tt=tt, c=c):
                        op_ps = pmx.tile([128, 512], f32, tag="mx", name="mx_ps")
                        for p in range(NP):
                            nc.tensor.matmul(
                                op_ps,
                                Onorm[p][:, tt * 128:(tt + 1) * 128],
                                WoSs[p][:, c * 512:(c + 1) * 512],
                                start=(p == 0),
                                stop=(p == 3),
                            )
                        oc = ocp_pool.tile([128, 512], f32, tag="ocp", name="oc")
                        nc.vector.tensor_copy(oc, op_ps)
                        nc.sync.dma_start(
                            out=out_d[tt * 128:(tt + 1) * 128,
                                      c * 512:(c + 1) * 512],
                            in_=oc)
                    thunks.append(unit)
            return thunks

        class PrevStage:
            def __init__(self, p, tq, exs):
                self.p, self.tq, self.exs = p, tq, exs
                self.acc = None   # [128 t, 512]: 8 x 64-wide accums, idx 4h+k
                self.den = None   # [128 t, 16]: cols 2k+h
                self.ots = None   # normalized [t, 128] tiles awaiting transpose

        def emit_pv_slot(prev, st):
            """PV for one s-tile of the previous stage: per (head h, t-tile
            k), a 64-wide main matmul (exp stationary, v moving) plus a
            1-wide denominator matmul against the ones column. One PSUM
            accumulation group per bank (lazy per-byte init)."""
            for h in range(2):
                vsl = vts[st][:, (2 * prev.p + h) * 64:(2 * prev.p + h + 1) * 64]
                for k in range(4):
                    idx = 4 * h + k
                    exsl = prev.exs[st][:, h * 512 + k * 128:
                                        h * 512 + (k + 1) * 128]
                    first = (st == 0 and idx == 0)
                    last = (st == NS - 1 and idx == 7)
                    nc.tensor.matmul(
                        prev.acc[:, idx * 64:(idx + 1) * 64],
                        exsl, vsl,
                        start=first, stop=last,
                    )
                    nc.tensor.matmul(
                        prev.den[:, 2 * k + h:2 * k + h + 1],
                        exsl, ones_sb,
                        start=first, stop=last,
                    )

        def emit_normalize_muls(prev):
            """Phase A of normalizing the previous stage's PV accumulators:
            one reciprocal over all 8 denominators, then per t-tile k a
            scalar-multiply of both heads' 64 cols into a [t, 128] bf16
            tile (DVE only — PE keeps streaming)."""
            rc = small.tile([128, 8], f32, tag="rc", name="rc")
            nc.vector.reciprocal(rc, prev.den[:, 0:8])
            ots = []
            for k in range(4):
                ot = otp.tile([128, 128], bf16, tag="ot", name="ot")
                for h in range(2):
                    nc.vector.tensor_scalar_mul(
                        ot[:, h * 64:(h + 1) * 64],
                        prev.acc[:, (4 * h + k) * 64:(4 * h + k + 1) * 64],
                        rc[:, 2 * k + h:2 * k + h + 1],
                    )
                ots.append(ot)
            return ots

        def emit_normalize_tp(prev, k):
            """Phase B (spread one per slot so the pmx ring and DVE copies
            never block PE): PE row-column transpose of one [t, 128] tile
            through a pmx slot, DVE copy into the [d, t]-major Onorm
            slice. Keeps normalize off the DMA queues entirely."""
            t0 = prev.tq * 4
            # ride the pacc ring: the acc tile was already consumed by the
            # muls, and the next stage's acc alloc waits for tp3's copy —
            # keeps transposes out of the busy pmx ring
            tp = pacc.tile([128, 128], bf16, tag="acc", name="tp_ps")
            nc.tensor.transpose(tp, prev.ots[k], ident_sb)
            nc.vector.tensor_copy(
                Onorm[prev.p][:, (t0 + k) * 128:(t0 + k + 1) * 128], tp)

        def emit_stage(p, tq, prev, extras, dl=6, pv_start=0):
            """16 score slots for (p, tq); interleave prev stage's PV at two
            PV-slots per score slot from `pv_start` (all its exp tiles
            already exist), normalizing mid-stage so the single PV
            accumulator bank clears well before the next stage reuses it;
            extra thunks all emitted by slot `dl`. Returns this stage's
            PrevStage record."""
            t0 = tq * 512
            exs = []
            if prev is not None:
                prev.acc = pacc.tile([128, 512], f32, tag="acc", name="acc_ps")
                prev.den = pden.tile([128, 16], f32, tag="den", name="den_ps")
            n_ex = len(extras)
            taken = 0
            pv_done = 0
            for st in range(NS):
                sc_ps = psc.tile([128, 1024], f32, tag="sc", name="sc_ps")
                nc.tensor.matmul(
                    sc_ps[:, 0:512],
                    kTs[p][0:64, st * 128:(st + 1) * 128],
                    qTs[p][0:64, t0:t0 + 512],
                    start=True, stop=True,
                    tile_position=(0, 0),
                )
                nc.tensor.matmul(
                    sc_ps[:, 512:1024],
                    kTs[p][64:128, st * 128:(st + 1) * 128],
                    qTs[p][64:128, t0:t0 + 512],
                    start=True, stop=True,
                    tile_position=(64, 0),
                )
                ex = expool.tile([128, 1024], bf16, tag="ex", name="ex")
                nc.scalar.activation(ex, sc_ps, AF.Exp, scale=0.125)
                exs.append(ex)
                if prev is not None and st >= pv_start:
                    if NS + 1 < pv_done < NS + 6:
                        emit_normalize_tp(prev, pv_done - NS - 2)
                        pv_done += 1
                    elif pv_done == NS + 1:
                        pv_done += 1   # one settle slot for the DVE muls
                    for _ in range(2):
                        if pv_done < NS:
                            emit_pv_slot(prev, pv_done)
                            pv_done += 1
                    if pv_done == NS:
                        prev.ots = emit_normalize_muls(prev)
                        pv_done = NS + 1
                want = (n_ex * min(st + 1, dl) + dl - 1) // dl
                while taken < want:
                    extras[taken]()
                    taken += 1
            while taken < n_ex:
                extras[taken]()
                taken += 1
            if prev is not None and pv_done < NS + 6:
                while pv_done < NS:
                    emit_pv_slot(prev, pv_done)
                    pv_done += 1
                if pv_done == NS:
                    prev.ots = emit_normalize_muls(prev)
                    pv_done = NS + 2
                elif pv_done == NS + 1:
                    pv_done = NS + 2
                while pv_done < NS + 6:
                    emit_normalize_tp(prev, pv_done - NS - 2)
                    pv_done += 1
            return PrevStage(p, tq, exs)

        # ---- emission ----
        for _rep in range(repeats):
            # startup critical path: pair-0 k weights, first k x-quarter
            # (split in two e-halves so MMs start after half the bytes),
            # then the same for q — only these five DMAs gate the first
            # score slot. Biases/WoS/ident/bv ride the idle ACT queue.
            # k-side criticals on the SP queue, q-side + constants on the
            # Pool queue (SWDGE — its own descriptor generator), so the two
            # startup streams issue in parallel.
            nc.gpsimd.dma_start(out=bqk_sb, in_=bqk_d[:, :])
            nc.sync.dma_start(out=wk2[:, 0:1024], in_=Wk0_d[:, :])
            k0 = proj_thunks(0, kT_d, wk2, kTs[0], bk_sb)
            q0 = proj_thunks(0, qT_d, wq2, qTs[0], bq_sb, eng=nc.gpsimd)
            k0[0]()
            nc.gpsimd.dma_start(out=wq2[:, 0:1024], in_=Wq0_d[:, :])
            q0[0]()
            k0[1](); k0[2](); k0[3]()
            q0[1](); q0[2](); q0[3]()
            # prefetch the stage-0/1 x stream before any bulk weight DMA;
            # kx quarter 1 rides Pool so its transfer follows the criticals
            k0[4]()   # kx quarter 1 (score slot 4, Pool queue)
            k0[8]()   # kx quarter 2 (score slot 8, SP queue)
            wv_tiles = load_wall(WvT_d)
            vpA = vproj_thunks(wv_tiles, 0)
            vpA[0]()  # vx quarter 0 (Pool queue)
            q0[4]()   # qx quarter 1 (stage 1, Pool queue)
            bv_ap = bv_d[:, :]
            bv_bcast_ap = bass.AP(
                tensor=bv_ap.tensor,
                offset=bv_ap.offset,
                ap=[[0, 128], bv_ap.ap[-1]],
            )
            nc.gpsimd.dma_start(out=bv_sb, in_=bv_bcast_ap)
            nc.gpsimd.dma_start(out=ident_sb, in_=ident_d[:, :])

            def late_weights():
                # pairs 1-3 weight walls: first use is stage 3 (~55us);
                # hold them past the startup/stage-0 DMA crunch
                with tc.tile_wait_until(0.028):
                    nc.sync.dma_start(out=wk2[:, 1024:4096], in_=WkR_d[:, :])
                    nc.sync.dma_start(out=wq2[:, 1024:4096], in_=WqR_d[:, :])

            # per-stage extra work, just-in-time:
            #  stage 0: k0 quarters 1-3 (score slots 4/8/12 need them),
            #           q0 quarter 1 (stage 1 start), vpA vst 0-7 + opens
            #  stage 1: vpA vst 8-15 (stage-1 PV consumes all vts by slot
            #           ~12), q0 quarter 2, pairs-1..3 weight walls
            #  stage 2: q0 quarter 3
            extras = {
                0: (k0[5:8] + k0[12:13] + k0[9:12] + vpA[5:6]
                    + k0[13:16] + vpA[1:5] + vpA[10:11]
                    + q0[5:8] + vpA[6:10] + vpA[15:16]),
                1: ([late_weights] + vpA[11:15] + vpA[16:20]
                    + q0[8:9] + q0[9:12]),
                2: q0[12:16],
            }
            # pair p>=1 just-in-time: x-quarter opens a stage ahead of
            # their MMs; k quarters 0/1 + q quarter 0 in stage 4p-1, k
            # quarters 2/3 (slot-8/12 deadlines) + q quarter 1 in stage
            # 4p, q quarters 2/3 in stages 4p+1/4p+2.
            for p in range(1, NP):
                qp = proj_thunks(p, qT_d, wq2, qTs[p], bq_sb)
                kp = proj_thunks(p, kT_d, wk2, kTs[p], bk_sb)
                for sg, th in (
                        (4 * p - 2, kp[0:1] + kp[4:5]),
                        (4 * p - 1, kp[1:4] + kp[5:8] + kp[8:9] + qp[0:1]
                         + kp[12:13] + qp[1:4]),
                        (4 * p, kp[9:12] + kp[13:16] + qp[4:5] + qp[5:8]),
                        (4 * p + 1, qp[8:9] + qp[9:12]),
                        (4 * p + 2, qp[12:13] + qp[13:16])):
                    extras[sg] = extras.get(sg, []) + th
            # second head-quad of V: vst(st) is consumed by stage 9's PV at
            # slot 5+st//2, so the tail quarters can ride the light stages
            # 6 and 9 — keeps every stage's extra-PE load ~3.4us.
            vpB = vproj_thunks(wv_tiles, 1)
            extras[2] = extras.get(2, []) + vpB[0:5]
            extras[3] = extras.get(3, []) + [load_late_inputs]
            extras[5] = extras.get(5, []) + vpB[5:10]
            extras[6] = extras.get(6, []) + vpB[10:15]
            extras[9] = extras.get(9, []) + vpB[15:20]
            # out-proj: pair-3 Onorm for tq0 lands ~slot 13 of stage 13
            # (PE-transpose path), so two op0 units fit at its very end;
            # the rest spread over stages 14/15.
            nops = [lambda: None] * 3
            op0 = outproj_thunks(0)
            extras[13] = extras.get(13, []) + nops + nops + op0[0:2]
            extras[14] = extras.get(14, []) + nops + op0[2:8]
            extras[15] = extras.get(15, []) + nops + outproj_thunks(1)

            # pacing: stage 1's vpA vsts feed stage-1 PV slots (dl=12
            # front-loads them); stage 9's vpB tail must be emitted before
            # the PV slots that read vts (slot 5+st//2); elsewhere even
            # spreading meets the k-quarter slot-8/12 deadlines.
            dls = {1: 12, 4: 10, 8: 10, 9: 7, 12: 10}
            pv_starts = {1: 5}
            prev = None
            for s in range(16):
                p, tq = s // 4, s % 4
                prev = emit_stage(p, tq, prev, extras.get(s, []),
                                  dl=dls.get(s, 16),
                                  pv_start=pv_starts.get(s, 1))

            # tail: PV of the last stage with out-proj(t2) interleaved
            # (its Onorm slices completed at the end of stage 15; hold the
            # last two op2 units to cover the final-normalize window), then
            # per t-tile: normalize on DVE, PE-transpose into a spare psc
            # slot, ACT-copy into Onorm — and out-proj(t3) as narrow units
            # whose PSUM drains ride the otherwise-idle ACT engine, each
            # unit's out-DMA issued as soon as it lands.
            prev.acc = pacc.tile([128, 512], f32, tag="acc", name="acc_ps")
            prev.den = pden.tile([128, 16], f32, tag="den", name="den_ps")
            op2 = outproj_thunks(2)
            for st in range(NS):
                emit_pv_slot(prev, st)
                if st % 2 == 1 and st // 2 < 6:
                    op2[st // 2]()
            rc = small.tile([128, 8], f32, tag="rc", name="rc")
            nc.vector.reciprocal(rc, prev.den[:, 0:8])
            held = op2[6:]
            ots = []
            for k in range(4):
                ot = otp.tile([128, 128], bf16, tag="ot", name="ot")
                for h in range(2):
                    nc.vector.tensor_scalar_mul(
                        ot[:, h * 64:(h + 1) * 64],
                        prev.acc[:, (4 * h + k) * 64:(4 * h + k + 1) * 64],
                        rc[:, 2 * k + h:2 * k + h + 1],
                    )
                ots.append(ot)

            def tail_tp(k):
                tp = psc.tile([128, 128], bf16, tag="sc", name="tp_ps")
                nc.tensor.transpose(tp, ots[k], ident_sb)
                nc.scalar.activation(
                    Onorm[3][:, (12 + k) * 128:(13 + k) * 128], tp, AF.Copy)

            def unit(tt, c):
                op_ps = pmx.tile([128, 512], f32, tag="mx", name="mx_ps")
                for p in range(NP):
                    nc.tensor.matmul(
                        op_ps,
                        Onorm[p][:, tt * 128:(tt + 1) * 128],
                        WoSs[p][:, c * 512:(c + 1) * 512],
                        start=(p == 0),
                        stop=(p == 3),
                    )
                oc = ocp_pool.tile([128, 512], f32, tag="ocp", name="oc")
                nc.scalar.activation(oc, op_ps, AF.Copy)
                eng = nc.sync if c == 0 else nc.scalar
                eng.dma_start(
                    out=out_d[tt * 128:(tt + 1) * 128,
                              c * 512:(c + 1) * 512],
                    in_=oc)

            # transposes run one t-tile ahead of their out-proj units so
            # PE never waits on the ACT Onorm copies
            tail_tp(0); held[0]()
            tail_tp(1); held[1]()
            unit(12, 0); unit(12, 1)
            tail_tp(2)
            unit(13, 0); unit(13, 1)
            tail_tp(3)
            unit(14, 0); unit(14, 1)
            unit(15, 0); unit(15, 1)

    nc.compile()
    return nc


def _get_nc():
    global _cached
    if _cached is None:
        _cached = _build()
    return _cached


def _prep_core_inputs(c, query, key, value, Wq, Wk, Wv, Wo, bq, bk, bv,
                      _cache={}):
    b, g = c // 2, c % 2
    sl = slice(g * DC, (g + 1) * DC)
    key_ = (id(query), b)
    if key_ not in _cache:
        # both cores of a batch share the transposed/cast activations
        _cache.clear()
        _cache[key_] = {
            "qT": query[b].T.astype(_BF16),
            "kT": key[b].T.astype(_BF16),
            "vT": value[b].T.astype(_BF16),
        }
    shared = _cache[key_]

    def pair_major(w):
        # [E, DC] -> per pair p: [128, 8*128] with (p*8+e)*128 indexing
        wt = w[sl].T.astype(_BF16)  # [E, DC]
        blocks = [
            np.ascontiguousarray(
                wt[:, p * 128:(p + 1) * 128]
                .reshape(8, 128, 128).transpose(1, 0, 2).reshape(128, 1024))
            for p in range(NP)
        ]
        return blocks[0], np.ascontiguousarray(np.concatenate(blocks[1:], 1))

    wq0, wqr = pair_major(Wq)
    wk0, wkr = pair_major(Wk)
    bqk = np.concatenate(
        [bq[sl].reshape(NP, 128).T, bk[sl].reshape(NP, 128).T], axis=1)
    return {
        **shared,
        "Wq0": wq0, "WqR": wqr,
        "Wk0": wk0, "WkR": wkr,
        "WvT": Wv[sl].T.astype(_BF16),
        "WoS": Wo[:, sl].T.astype(_BF16),
        "bqk": np.ascontiguousarray(bqk),
        "bv": np.ascontiguousarray(bv[sl].reshape(1, DC)),
        "ident": np.eye(128, dtype=_BF16),
    }


def kernel(**inputs):
    from concourse.bass_utils import run_bass_kernel_spmd

    args = {k: np.asarray(inputs[k], np.float32)
            for k in ("query", "key", "value", "Wq", "Wk", "Wv", "Wo",
                      "bq", "bk", "bv", "bo")}
    _prep_core_inputs.__defaults__[0].clear()
    nc = _get_nc()
    in_maps = [
        _prep_core_inputs(c, args["query"], args["key"], args["value"],
                          args["Wq"], args["Wk"], args["Wv"], args["Wo"],
                          args["bq"], args["bk"], args["bv"])
        for c in range(8)
    ]
    res = run_bass_kernel_spmd(nc, in_maps, core_ids=list(range(8)))
    outs = [r["out"] for r in res.results]
    final = np.empty((B, T, E), np.float32)
    for b in range(B):
        final[b] = outs[2 * b] + outs[2 * b + 1] + args["bo"][None, :]
    return final

